# revision 21
# baseline (speedup 1.0000x reference)
"""Trainium2 Bass kernel for pairwise DiceLoss.

Math (per reference):
    an[b,k,:]  = am[b,k,:] / (S[b,k] + EPS),  S = row sums of am
    gram_n     = an . an^T per batch          (K x K per batch)
    dice[b,k,l]= (2*gram_n + 0.1) / (a[b,k] + a[b,l] + 0.1),  a = S/(S+EPS)
    loss       = mean over b of dice, masked to k<l pairs, then mean over pairs

Since a = S/(S+1e-8) = 1 - eps/(S+eps) and S ~ 3e4 here, the denominator is
2.1 to within ~1e-13 relative (far below f32 ulp) -> den is the constant 2.1
and the 0.1 numerator term reduces to a host-side constant (+1/21 on the
final mean).  What remains on-device per core is the Gram of its rows and a
tiny masked weighted reduction.

Heavy part: per-batch Gram of a 16 x 65536 matrix -> one full pass over the
input. 8 batches/core x 16 slots = 128 rows = the 128 SBUF partitions; host
appends a ones-row so row sums fall out of the Gram as one extra rhs column.
Host quantizes to fp8e4m3 (4x less HBM traffic; f32 PSUM accumulate keeps
the quantization error ~1e-9 after averaging) and pre-arranges to
[p, c, bk] so DMAs land contiguous and matmul operands are contiguous
(n = p*512 + c is a pure relabeling of the contraction index).

Schedule (per core):
  - A memset fp8 tile feeds ~20 dummy warm-up matmuls so the PE's HAM clock
    gate reaches 2.4 GHz while the first x tiles are still in flight (the PE
    otherwise runs its first ~3.4us at 1.2 GHz).
  - x tile DMAs are issued round-robin from sync/scalar/gpsimd so the
    ~650ns DMA_DIRECT2D issue cost doesn't serialize on one engine.
  - 512 accumulating PE matmuls (lhsT = chunk [128p x 128bk], rhs = chunk +
    ones col, N=129) -> PSUM [128,129] = cross-Gram + row sums.
  - Epilogue: r = 1/(S+eps); GM2 = G * mask21 (mask21[m,j] = (2/2.1) * same
    batch-block upper-tri mask, f32 consts); one tiny N=1 f32 matmul
    out[m] = sum_j GM2[j,m]*r[j] (mask transpose is fine: the final total is
    symmetric in (m,j)); loss_m = r_m * out[m]; DMA out [128,1].
Host: loss = sum(out)/(64*120) + 1/21.

Baseline (prev session) was 53.8us; this version targets ~36us: the 512-MM
stream (~29us warm) is simultaneously at the PE streaming floor (1 col/cycle)
and near the HBM floor (8.45MB @ ~320GB/s), so the wins are warm-up, DMA
issue parallelism, and a short epilogue.
"""

import os

import numpy as np

B, K, N = 64, 16, 65536
NCORES = 8
BPC = B // NCORES  # batches per core
R = BPC * K  # 128 data rows per core
P = 128  # SBUF partitions
C_PER_P = N // P  # 512 columns per row after [p, c] reshape
# small first tiles -> PE starts early; bigger later tiles have big
# per-partition DMA packets (the DMA ring is packet-rate bound below ~8KB
# packets, ~420GB/s above). Sums to C_PER_P.
TILES = [8, 8, 16, 24, 48, 80, 108, 112, 108]
SMOOTH = 0.1
EPS = 1e-8
WARMUP_MMS = int(os.environ.get("KERNEL_WARMUP", "22"))

_CACHE: dict = {}

# test.py reads this after calling kernel() to print HW exec time
LAST_RESULTS = None


def _build_nc():
    import concourse.bacc as bacc
    import concourse.mybir as mybir
    import concourse.tile as tile

    f32 = mybir.dt.float32
    xdt = mybir.dt.float8e4
    alu = mybir.AluOpType
    nc = bacc.Bacc("TRN2", target_bir_lowering=False)

    x = nc.dram_tensor("x", [P, C_PER_P, R + 1], xdt, kind="ExternalInput")
    consts = nc.dram_tensor("consts", [P, P], f32, kind="ExternalInput")
    out_g = nc.dram_tensor("out_g", [1, 1], f32, kind="ExternalOutput")

    with tile.TileContext(nc) as tc:
        with (
            tc.tile_pool(name="xp", bufs=1) as xp,
            tc.tile_pool(name="sg", bufs=1) as sg,
            tc.tile_pool(name="ps", bufs=1, space="PSUM") as ps,
            tc.tile_pool(name="ps2", bufs=1, space="PSUM") as ps2,
            tc.tile_pool(name="psw", bufs=1, space="PSUM") as psw,
        ):
            g_ps = ps.tile([P, R + 1], f32)

            m21_sb = sg.tile([P, P], f32)

            # ---- PE warm-up: memset junk fp8, matmul it while DMAs fly ----
            warm_sb = sg.tile([P, P], xdt)
            nc.gpsimd.memset(warm_sb[:], 1.0)
            warm_ps = psw.tile([P, P], f32)
            for i in range(WARMUP_MMS):
                nc.tensor.matmul(
                    warm_ps[:], warm_sb[:], warm_sb[:], start=True, stop=True
                )

            # ---- input DMAs: all on sync -> one DMA ring, in-order, full
            # bandwidth (per-engine rings split bandwidth ~3x, measured) ----
            xts = []
            off = 0
            for t, cc in enumerate(TILES):
                xt = xp.tile([P, cc, R + 1], xdt, name=f"xt{t}")
                nc.sync.dma_start(xt[:], x[:, off : off + cc, :])
                xts.append(xt)
                off += cc
            # mask consts ride the same ring after the x tiles: needed only
            # by the epilogue, so they're never on the critical path and
            # don't delay any x tile.
            nc.sync.dma_start(m21_sb[:], consts[:, :])

            # ---- the Gram stream: 512 accumulating matmuls, N=129 ----
            ntot = sum(TILES)
            mm = 0
            for t, cc in enumerate(TILES):
                xt = xts[t]
                for c in range(cc):
                    nc.tensor.matmul(
                        g_ps[:],
                        xt[:, c, 0:R],
                        xt[:, c, :],
                        start=(mm == 0),
                        stop=(mm == ntot - 1),
                    )
                    mm += 1

            # ---- epilogue: total = r^T (G o m21) r, down to ONE scalar ----
            # (a [128,1] output DMA is 128 four-byte packets and costs ~7us
            # in ring latency; a [1,1] scalar is a single packet)
            s_ps = g_ps[:, R : R + 1]  # S[row], in PSUM
            seps = sg.tile([P, 2], f32)
            rcol = seps[:, 0:1]
            # S ~ 3e4 so the reference's +1e-8 is below f32 ulp -> skip it
            nc.vector.reciprocal(rcol, s_ps)  # r = 1/S

            # GM2[j, m] = G[j, m] * m21[j, m]  (f32, SBUF) -- used as lhsT
            gm2 = sg.tile([P, P], f32)
            nc.vector.tensor_mul(gm2[:], g_ps[:, 0:R], m21_sb[:])
            # dsum[m] = sum_j GM2[j, m] * r[j]   (tiny N=1 f32 matmul)
            dsum_ps = ps2.tile([P, 1], f32)
            nc.tensor.matmul(dsum_ps[:], gm2[:], rcol, start=True, stop=True)
            dsum_sb = sg.tile([P, 1], f32)
            nc.vector.tensor_copy(out=dsum_sb[:], in_=dsum_ps[:])
            # tot = sum_m dsum[m] * r[m]
            tot_ps = ps2.tile([1, 1], f32)
            nc.tensor.matmul(tot_ps[:], dsum_sb[:], rcol, start=True, stop=True)
            osb = sg.tile([1, 1], f32)
            nc.vector.tensor_copy(out=osb[:], in_=tot_ps[:])
            nc.sync.dma_start(out_g[:, :], osb[:])

    nc.compile()
    return nc


def _make_consts() -> np.ndarray:
    # m21[m, j] = 2/2.1 iff same batch block and k < l, else 0
    m = np.arange(P)[:, None]
    j = np.arange(P)[None, :]
    mask = (m // K == j // K) & (m % K < j % K)
    return np.where(mask, np.float32(2.0 / 2.1), np.float32(0.0))


def _shard_core(am_rows: np.ndarray) -> np.ndarray:
    """[128, 65536] f32 -> [P, CC, 129] fp8 device layout (+ ones row)."""
    import ml_dtypes

    xr = np.empty((R + 1, N), dtype=ml_dtypes.float8_e4m3)
    xr[:R] = am_rows.astype(ml_dtypes.float8_e4m3)
    xr[R] = 1.0
    # n = p*512 + c ; [bk, p, c] -> [p, c, bk]
    xt = xr.reshape(R + 1, P, C_PER_P).transpose(1, 2, 0)
    return np.ascontiguousarray(xt)


def kernel(am: np.ndarray) -> np.ndarray:
    global LAST_RESULTS
    from concourse.bass_utils import run_bass_kernel_spmd

    if "nc" not in _CACHE:
        _CACHE["nc"] = _build_nc()
        _CACHE["consts"] = _make_consts()
    nc = _CACHE["nc"]
    consts = _CACHE["consts"]

    am = np.ascontiguousarray(np.asarray(am), dtype=np.float32)
    assert am.shape == (B, K, N)

    in_maps = []
    for core in range(NCORES):
        rows = am[core * BPC : (core + 1) * BPC].reshape(R, N)
        in_maps.append({"x": _shard_core(rows), "consts": consts})

    trace = bool(int(os.environ.get("KERNEL_TRACE", "0")))
    res = run_bass_kernel_spmd(
        nc, in_maps, core_ids=list(range(NCORES)), trace=trace
    )
    LAST_RESULTS = res

    total = float(
        np.sum(
            np.array([r["out_g"][0, 0] for r in res.results], dtype=np.float64)
        )
    )
    npairs = K * (K - 1) // 2
    # dice = (2*gram_n + S)/2.1: the gram part is `total`, the +S/2.1 part
    # is constant per masked pair -> + S/2.1 per pair = +1/21 on the mean.
    return np.float32(total / (B * npairs) + SMOOTH / 2.1)


# revision 24
# speedup vs baseline: 1.0292x; 1.0292x over previous
"""Trainium2 Bass kernel for pairwise DiceLoss.

Math (per reference):
    an[b,k,:]  = am[b,k,:] / (S[b,k] + EPS),  S = row sums of am
    gram_n     = an . an^T per batch          (K x K per batch)
    dice[b,k,l]= (2*gram_n + 0.1) / (a[b,k] + a[b,l] + 0.1),  a = S/(S+EPS)
    loss       = mean over b of dice, masked to k<l pairs, then mean over pairs

Since a = S/(S+1e-8) = 1 - eps/(S+eps) and S ~ 3e4 here, the denominator is
2.1 to within ~1e-13 relative (far below f32 ulp) -> den is the constant 2.1
and the 0.1 numerator term reduces to a host-side constant (+1/21 on the
final mean).  What remains on-device per core is the Gram of its rows and a
tiny masked weighted reduction.

Heavy part: per-batch Gram of a 16 x 65536 matrix -> one full pass over the
input. 8 batches/core x 16 slots = 128 rows = the 128 SBUF partitions; host
appends a ones-row so row sums fall out of the Gram as one extra rhs column.
Host quantizes to fp8e4m3 (4x less HBM traffic; f32 PSUM accumulate keeps
the quantization error ~1e-9 after averaging) and pre-arranges to
[p, c, bk] so DMAs land contiguous and matmul operands are contiguous
(n = p*512 + c is a pure relabeling of the contraction index).

Schedule (per core):
  - A memset fp8 tile feeds ~20 dummy warm-up matmuls so the PE's HAM clock
    gate reaches 2.4 GHz while the first x tiles are still in flight (the PE
    otherwise runs its first ~3.4us at 1.2 GHz).
  - x tile DMAs are issued round-robin from sync/scalar/gpsimd so the
    ~650ns DMA_DIRECT2D issue cost doesn't serialize on one engine.
  - 512 accumulating PE matmuls (lhsT = chunk [128p x 128bk], rhs = chunk +
    ones col, N=129) -> PSUM [128,129] = cross-Gram + row sums.
  - Epilogue: r = 1/(S+eps); GM2 = G * mask21 (mask21[m,j] = (2/2.1) * same
    batch-block upper-tri mask, f32 consts); one tiny N=1 f32 matmul
    out[m] = sum_j GM2[j,m]*r[j] (mask transpose is fine: the final total is
    symmetric in (m,j)); loss_m = r_m * out[m]; DMA out [128,1].
Host: loss = sum(out)/(64*120) + 1/21.

Baseline (prev session) was 53.8us; this version targets ~36us: the 512-MM
stream (~29us warm) is simultaneously at the PE streaming floor (1 col/cycle)
and near the HBM floor (8.45MB @ ~320GB/s), so the wins are warm-up, DMA
issue parallelism, and a short epilogue.
"""

import os

import numpy as np

B, K, N = 64, 16, 65536
NCORES = 8
BPC = B // NCORES  # batches per core
R = BPC * K  # 128 data rows per core
P = 128  # SBUF partitions
C_PER_P = N // P  # 512 columns per row after [p, c] reshape
# small first tiles -> PE starts early; bigger later tiles have big
# per-partition DMA packets (the DMA ring is packet-rate bound below ~8KB
# packets, ~420GB/s above). Sums to C_PER_P.
TILES = [8, 8, 16, 32, 64, 96, 108, 92, 88]
SMOOTH = 0.1
EPS = 1e-8
WARMUP_MMS = int(os.environ.get("KERNEL_WARMUP", "22"))

_CACHE: dict = {}

# test.py reads this after calling kernel() to print HW exec time
LAST_RESULTS = None


def _build_nc():
    import concourse.bacc as bacc
    import concourse.mybir as mybir
    import concourse.tile as tile

    f32 = mybir.dt.float32
    xdt = mybir.dt.float8e4
    alu = mybir.AluOpType
    nc = bacc.Bacc("TRN2", target_bir_lowering=False)

    x = nc.dram_tensor("x", [P, C_PER_P, R + 1], xdt, kind="ExternalInput")
    consts = nc.dram_tensor("consts", [P, P], f32, kind="ExternalInput")
    out_g = nc.dram_tensor("out_g", [1, 1], f32, kind="ExternalOutput")

    with tile.TileContext(nc) as tc:
        with (
            tc.tile_pool(name="xp", bufs=1) as xp,
            tc.tile_pool(name="sg", bufs=1) as sg,
            tc.tile_pool(name="ps", bufs=1, space="PSUM") as ps,
            tc.tile_pool(name="ps2", bufs=1, space="PSUM") as ps2,
            tc.tile_pool(name="psw", bufs=1, space="PSUM") as psw,
        ):
            g_ps = ps.tile([P, R + 1], f32)

            m21_sb = sg.tile([P, P], f32)

            # ---- PE warm-up: memset junk fp8, matmul it while DMAs fly ----
            warm_sb = sg.tile([P, P], xdt)
            nc.gpsimd.memset(warm_sb[:], 1.0)
            warm_ps = psw.tile([P, P], f32)
            for i in range(WARMUP_MMS):
                nc.tensor.matmul(
                    warm_ps[:], warm_sb[:], warm_sb[:], start=True, stop=True
                )

            # ---- input DMAs: all on sync -> one DMA ring, in-order, full
            # bandwidth (per-engine rings split bandwidth ~3x, measured) ----
            xts = []
            off = 0
            for t, cc in enumerate(TILES):
                xt = xp.tile([P, cc, R + 1], xdt, name=f"xt{t}")
                nc.sync.dma_start(xt[:], x[:, off : off + cc, :])
                xts.append(xt)
                off += cc
            # mask consts ride the same ring after the x tiles: needed only
            # by the epilogue, so they're never on the critical path and
            # don't delay any x tile.
            nc.sync.dma_start(m21_sb[:], consts[:, :])

            # ---- the Gram stream: 512 accumulating matmuls, N=129 ----
            ntot = sum(TILES)
            mm = 0
            for t, cc in enumerate(TILES):
                xt = xts[t]
                for c in range(cc):
                    nc.tensor.matmul(
                        g_ps[:],
                        xt[:, c, 0:R],
                        xt[:, c, :],
                        start=(mm == 0),
                        stop=(mm == ntot - 1),
                    )
                    mm += 1

            # ---- epilogue: total = r^T (G o m21) r, down to ONE scalar ----
            # (a [128,1] output DMA is 128 four-byte packets and costs ~7us
            # in ring latency; a [1,1] scalar is a single packet)
            s_ps = g_ps[:, R : R + 1]  # S[row], in PSUM
            bf16 = mybir.dt.bfloat16
            seps = sg.tile([P, 2], bf16)
            rcol = seps[:, 0:1]
            # S ~ 3e4 so the reference's +1e-8 is below f32 ulp -> skip it.
            # bf16 r/gm2: single-pass PE matmuls + FWL weight loads; the
            # ~0.4% per-element rounding averages out to ~1e-4 on the loss
            # (gate 2e-2).
            with nc.allow_low_precision("bf16 epilogue, ~1e-4 vs 2e-2 gate"):
                nc.vector.reciprocal(rcol, s_ps)  # r = 1/S

                # GM2[j, m] = G[j, m] * m21[j, m]  -- used as lhsT
                gm2 = sg.tile([P, P], bf16)
                nc.vector.tensor_mul(gm2[:], g_ps[:, 0:R], m21_sb[:])
            # dsum[m] = sum_j GM2[j, m] * r[j]   (tiny N=1 matmul)
            dsum_ps = ps2.tile([P, 1], f32)
            nc.tensor.matmul(dsum_ps[:], gm2[:], rcol, start=True, stop=True)
            dsum_sb = sg.tile([P, 1], bf16)
            with nc.allow_low_precision("bf16 epilogue, ~1e-4 vs 2e-2 gate"):
                nc.vector.tensor_copy(out=dsum_sb[:], in_=dsum_ps[:])
            # tot = sum_m dsum[m] * r[m]
            tot_ps = ps2.tile([1, 1], f32)
            nc.tensor.matmul(tot_ps[:], dsum_sb[:], rcol, start=True, stop=True)
            osb = sg.tile([1, 1], f32)
            nc.vector.tensor_copy(out=osb[:], in_=tot_ps[:])
            nc.sync.dma_start(out_g[:, :], osb[:])

    nc.compile()
    return nc


def _make_consts() -> np.ndarray:
    # m21[m, j] = 2/2.1 iff same batch block and k < l, else 0
    m = np.arange(P)[:, None]
    j = np.arange(P)[None, :]
    mask = (m // K == j // K) & (m % K < j % K)
    return np.where(mask, np.float32(2.0 / 2.1), np.float32(0.0))


def _shard_core(am_rows: np.ndarray) -> np.ndarray:
    """[128, 65536] f32 -> [P, CC, 129] fp8 device layout (+ ones row)."""
    import ml_dtypes

    xr = np.empty((R + 1, N), dtype=ml_dtypes.float8_e4m3)
    xr[:R] = am_rows.astype(ml_dtypes.float8_e4m3)
    xr[R] = 1.0
    # n = p*512 + c ; [bk, p, c] -> [p, c, bk]
    xt = xr.reshape(R + 1, P, C_PER_P).transpose(1, 2, 0)
    return np.ascontiguousarray(xt)


def kernel(am: np.ndarray) -> np.ndarray:
    global LAST_RESULTS
    from concourse.bass_utils import run_bass_kernel_spmd

    if "nc" not in _CACHE:
        _CACHE["nc"] = _build_nc()
        _CACHE["consts"] = _make_consts()
    nc = _CACHE["nc"]
    consts = _CACHE["consts"]

    am = np.ascontiguousarray(np.asarray(am), dtype=np.float32)
    assert am.shape == (B, K, N)

    in_maps = []
    for core in range(NCORES):
        rows = am[core * BPC : (core + 1) * BPC].reshape(R, N)
        in_maps.append({"x": _shard_core(rows), "consts": consts})

    trace = bool(int(os.environ.get("KERNEL_TRACE", "0")))
    res = run_bass_kernel_spmd(
        nc, in_maps, core_ids=list(range(NCORES)), trace=trace
    )
    LAST_RESULTS = res

    total = float(
        np.sum(
            np.array([r["out_g"][0, 0] for r in res.results], dtype=np.float64)
        )
    )
    npairs = K * (K - 1) // 2
    # dice = (2*gram_n + S)/2.1: the gram part is `total`, the +S/2.1 part
    # is constant per masked pair -> + S/2.1 per pair = +1/21 on the mean.
    return np.float32(total / (B * npairs) + SMOOTH / 2.1)


# revision 25
# speedup vs baseline: 1.0389x; 1.0094x over previous
"""Trainium2 Bass kernel for pairwise DiceLoss.

Math (per reference):
    an[b,k,:]  = am[b,k,:] / (S[b,k] + EPS),  S = row sums of am
    gram_n     = an . an^T per batch          (K x K per batch)
    dice[b,k,l]= (2*gram_n + 0.1) / (a[b,k] + a[b,l] + 0.1),  a = S/(S+EPS)
    loss       = mean over b of dice, masked to k<l pairs, then mean over pairs

Since a = S/(S+1e-8) = 1 - eps/(S+eps) and S ~ 3e4 here, the denominator is
2.1 to within ~1e-13 relative (far below f32 ulp) -> den is the constant 2.1
and the 0.1 numerator term reduces to a host-side constant (+1/21 on the
final mean).  What remains on-device per core is the Gram of its rows and a
tiny masked weighted reduction.

Heavy part: per-batch Gram of a 16 x 65536 matrix -> one full pass over the
input. 8 batches/core x 16 slots = 128 rows = the 128 SBUF partitions; host
appends a ones-row so row sums fall out of the Gram as one extra rhs column.
Host quantizes to fp8e4m3 (4x less HBM traffic; f32 PSUM accumulate keeps
the quantization error ~1e-9 after averaging) and pre-arranges to
[p, c, bk] so DMAs land contiguous and matmul operands are contiguous
(n = p*512 + c is a pure relabeling of the contraction index).

Schedule (per core):
  - A memset fp8 tile feeds ~20 dummy warm-up matmuls so the PE's HAM clock
    gate reaches 2.4 GHz while the first x tiles are still in flight (the PE
    otherwise runs its first ~3.4us at 1.2 GHz).
  - x tile DMAs are issued round-robin from sync/scalar/gpsimd so the
    ~650ns DMA_DIRECT2D issue cost doesn't serialize on one engine.
  - 512 accumulating PE matmuls (lhsT = chunk [128p x 128bk], rhs = chunk +
    ones col, N=129) -> PSUM [128,129] = cross-Gram + row sums.
  - Epilogue: r = 1/(S+eps); GM2 = G * mask21 (mask21[m,j] = (2/2.1) * same
    batch-block upper-tri mask, f32 consts); one tiny N=1 f32 matmul
    out[m] = sum_j GM2[j,m]*r[j] (mask transpose is fine: the final total is
    symmetric in (m,j)); loss_m = r_m * out[m]; DMA out [128,1].
Host: loss = sum(out)/(64*120) + 1/21.

Baseline (prev session) was 53.8us; this version targets ~36us: the 512-MM
stream (~29us warm) is simultaneously at the PE streaming floor (1 col/cycle)
and near the HBM floor (8.45MB @ ~320GB/s), so the wins are warm-up, DMA
issue parallelism, and a short epilogue.
"""

import os

import numpy as np

B, K, N = 64, 16, 65536
NCORES = 8
BPC = B // NCORES  # batches per core
R = BPC * K  # 128 data rows per core
P = 128  # SBUF partitions
C_PER_P = N // P  # 512 columns per row after [p, c] reshape
# small first tiles -> PE starts early; bigger later tiles have big
# per-partition DMA packets (the DMA ring is packet-rate bound below ~8KB
# packets, ~420GB/s above). Sums to C_PER_P.
TILES = [4, 8, 12, 32, 64, 96, 108, 96, 92]
SMOOTH = 0.1
EPS = 1e-8
WARMUP_MMS = int(os.environ.get("KERNEL_WARMUP", "22"))

_CACHE: dict = {}

# test.py reads this after calling kernel() to print HW exec time
LAST_RESULTS = None


def _build_nc():
    import concourse.bacc as bacc
    import concourse.mybir as mybir
    import concourse.tile as tile

    f32 = mybir.dt.float32
    xdt = mybir.dt.float8e4
    alu = mybir.AluOpType
    nc = bacc.Bacc("TRN2", target_bir_lowering=False)

    x = nc.dram_tensor("x", [P, C_PER_P, R + 1], xdt, kind="ExternalInput")
    consts = nc.dram_tensor("consts", [P, P], f32, kind="ExternalInput")
    out_g = nc.dram_tensor("out_g", [1, 1], f32, kind="ExternalOutput")

    with tile.TileContext(nc) as tc:
        with (
            tc.tile_pool(name="xp", bufs=1) as xp,
            tc.tile_pool(name="sg", bufs=1) as sg,
            tc.tile_pool(name="ps", bufs=1, space="PSUM") as ps,
            tc.tile_pool(name="ps2", bufs=1, space="PSUM") as ps2,
            tc.tile_pool(name="psw", bufs=1, space="PSUM") as psw,
        ):
            g_ps = ps.tile([P, R + 1], f32)

            m21_sb = sg.tile([P, P], f32)

            # ---- PE warm-up: memset junk fp8, matmul it while DMAs fly ----
            warm_sb = sg.tile([P, P], xdt)
            nc.gpsimd.memset(warm_sb[:], 1.0)
            warm_ps = psw.tile([P, P], f32)
            for i in range(WARMUP_MMS):
                nc.tensor.matmul(
                    warm_ps[:], warm_sb[:], warm_sb[:], start=True, stop=True
                )

            # ---- input DMAs: all on sync -> one DMA ring, in-order, full
            # bandwidth (per-engine rings split bandwidth ~3x, measured) ----
            xts = []
            off = 0
            for t, cc in enumerate(TILES):
                xt = xp.tile([P, cc, R + 1], xdt, name=f"xt{t}")
                nc.sync.dma_start(xt[:], x[:, off : off + cc, :])
                xts.append(xt)
                off += cc
            # mask consts ride the same ring after the x tiles: needed only
            # by the epilogue, so they're never on the critical path and
            # don't delay any x tile.
            nc.sync.dma_start(m21_sb[:], consts[:, :])

            # ---- the Gram stream: 512 accumulating matmuls, N=129 ----
            ntot = sum(TILES)
            mm = 0
            for t, cc in enumerate(TILES):
                xt = xts[t]
                for c in range(cc):
                    nc.tensor.matmul(
                        g_ps[:],
                        xt[:, c, 0:R],
                        xt[:, c, :],
                        start=(mm == 0),
                        stop=(mm == ntot - 1),
                    )
                    mm += 1

            # ---- epilogue: total = r^T (G o m21) r, down to ONE scalar ----
            # (a [128,1] output DMA is 128 four-byte packets and costs ~7us
            # in ring latency; a [1,1] scalar is a single packet)
            s_ps = g_ps[:, R : R + 1]  # S[row], in PSUM
            bf16 = mybir.dt.bfloat16
            seps = sg.tile([P, 2], bf16)
            rcol = seps[:, 0:1]
            # S ~ 3e4 so the reference's +1e-8 is below f32 ulp -> skip it.
            # bf16 r/gm2: single-pass PE matmuls + FWL weight loads; the
            # ~0.4% per-element rounding averages out to ~1e-4 on the loss
            # (gate 2e-2).
            with nc.allow_low_precision("bf16 epilogue, ~1e-4 vs 2e-2 gate"):
                nc.vector.reciprocal(rcol, s_ps)  # r = 1/S

                # GM2[j, m] = G[j, m] * m21[j, m]  -- used as lhsT
                gm2 = sg.tile([P, P], bf16)
                nc.vector.tensor_mul(gm2[:], g_ps[:, 0:R], m21_sb[:])
            # dsum[m] = sum_j GM2[j, m] * r[j]   (tiny N=1 matmul)
            dsum_ps = ps2.tile([P, 1], f32)
            nc.tensor.matmul(dsum_ps[:], gm2[:], rcol, start=True, stop=True)
            dsum_sb = sg.tile([P, 1], bf16)
            with nc.allow_low_precision("bf16 epilogue, ~1e-4 vs 2e-2 gate"):
                nc.vector.tensor_copy(out=dsum_sb[:], in_=dsum_ps[:])
            # tot = sum_m dsum[m] * r[m]
            tot_ps = ps2.tile([1, 1], f32)
            nc.tensor.matmul(tot_ps[:], dsum_sb[:], rcol, start=True, stop=True)
            osb = sg.tile([1, 1], f32)
            nc.vector.tensor_copy(out=osb[:], in_=tot_ps[:])
            nc.sync.dma_start(out_g[:, :], osb[:])

    nc.compile()
    return nc


def _make_consts() -> np.ndarray:
    # m21[m, j] = 2/2.1 iff same batch block and k < l, else 0
    m = np.arange(P)[:, None]
    j = np.arange(P)[None, :]
    mask = (m // K == j // K) & (m % K < j % K)
    return np.where(mask, np.float32(2.0 / 2.1), np.float32(0.0))


def _shard_core(am_rows: np.ndarray) -> np.ndarray:
    """[128, 65536] f32 -> [P, CC, 129] fp8 device layout (+ ones row)."""
    import ml_dtypes

    xr = np.empty((R + 1, N), dtype=ml_dtypes.float8_e4m3)
    xr[:R] = am_rows.astype(ml_dtypes.float8_e4m3)
    xr[R] = 1.0
    # n = p*512 + c ; [bk, p, c] -> [p, c, bk]
    xt = xr.reshape(R + 1, P, C_PER_P).transpose(1, 2, 0)
    return np.ascontiguousarray(xt)


def kernel(am: np.ndarray) -> np.ndarray:
    global LAST_RESULTS
    from concourse.bass_utils import run_bass_kernel_spmd

    if "nc" not in _CACHE:
        _CACHE["nc"] = _build_nc()
        _CACHE["consts"] = _make_consts()
    nc = _CACHE["nc"]
    consts = _CACHE["consts"]

    am = np.ascontiguousarray(np.asarray(am), dtype=np.float32)
    assert am.shape == (B, K, N)

    in_maps = []
    for core in range(NCORES):
        rows = am[core * BPC : (core + 1) * BPC].reshape(R, N)
        in_maps.append({"x": _shard_core(rows), "consts": consts})

    trace = bool(int(os.environ.get("KERNEL_TRACE", "0")))
    res = run_bass_kernel_spmd(
        nc, in_maps, core_ids=list(range(NCORES)), trace=trace
    )
    LAST_RESULTS = res

    total = float(
        np.sum(
            np.array([r["out_g"][0, 0] for r in res.results], dtype=np.float64)
        )
    )
    npairs = K * (K - 1) // 2
    # dice = (2*gram_n + S)/2.1: the gram part is `total`, the +S/2.1 part
    # is constant per masked pair -> + S/2.1 per pair = +1/21 on the mean.
    return np.float32(total / (B * npairs) + SMOOTH / 2.1)
